# revision 5
# baseline (speedup 1.0000x reference)
"""GCN layer on 8 TRN2 NeuronCores (Bass/Tile).

out = segment_sum(edge_vals[:,None] * (X @ W)[edge_col], edge_row, N)

Strategy (1D destination-node sharding, v4):
  - Host: cast/transpose X -> XT bf16 and hand each core only ITS slice of
    the projection work: core p computes pair-tiles a in [25p, 25(p+1)) of a
    200-pair-tile table (HALF=25600; table row r packs node r in cols 0:64
    and node r+HALF in cols 64:128 -> 256 B rows, int16-indexable).
  - Device phase 1 (sharded 8x): each core projects its 6400 nodes
    (2 x 25 tiles x 128) through W into a [128, 3200] staging tile, DMAs it
    to a local scratch, then an AllGather collective assembles the full
    [25600, 128] bf16 table on every core. Phase-1 HBM traffic drops from
    32 MB to ~12 MB per core.
  - Device phase 2: dma_gather (SWDGE) fetches table rows per edge across 4
    queues (CH=32 tiles per call to amortize the ~1us per-call fixed cost);
    VectorE builds S[e,r] = (row_local[e] == r) via an iota compare, then
    SE = S*valsE and SO = S*valsO (all-bf16 operands with unit inner stride
    -> DVE 2x mode; vals broadcast over the middle dim). TensorE accumulates
    SE^T @ g[:,:,0:64] + SO^T @ g[:,:,64:128] into the window's PSUM
    [128, 64]. ScalarE copies PSUM to SBUF for the output DMA.
  - Host: concatenate the 8 output shards.
"""

from contextlib import ExitStack

import ml_dtypes
import numpy as np

import concourse.bacc as bacc
import concourse.bass as bass
import concourse.mybir as mybir
import concourse.tile as tile
from concourse._compat import get_trn_type
from concourse.bass_utils import run_bass_kernel_spmd

N_NODES = 50000
N_EDGES = 800000
F_IN = 256
F_OUT = 64
N_CORES = 8
SHARD = N_NODES // N_CORES  # 6250 destination rows per core
WIN = 128  # destination rows per PSUM accumulation window
BF16 = ml_dtypes.bfloat16

HALF = 25600  # node-pair split: row r holds node r (cols 0:64) and r+HALF
NTL = HALF // 128  # 200 pair-tiles; table rows = HALF, all int16-addressable
PT = NTL // N_CORES  # 25 pair-tiles computed per core in phase 1

# knobs
CH = 32  # phase-2 edge tiles (of 128 edges) per dma_gather call
GB = 16  # phase-2 edge tiles per batched one-hot / scale build (divides CH)
NQ = 4  # SWDGE queues used round-robin for gather desc-gen

# test.py pokes these for profiling
TRACE = False
LAST_RESULTS = None


def _install_ntff_hook():
    """The agent image's antenv lacks axon_hooks, so bass_utils' trace=True
    path can't find the NTFF hook. Recreate the module and register the
    ctypes-based hook exactly as trn_agent_boot would."""
    import sys
    import types

    try:
        import antenv.axon_hooks  # noqa: F401

        return True
    except ImportError:
        pass
    try:
        import antenv
        from trn_agent_boot.trn_boot import _ntff_profile_via_ctypes

        mod = types.ModuleType("antenv.axon_hooks")
        mod._hook = None

        def set_axon_ntff_profile_hook(h):
            mod._hook = h

        def get_axon_ntff_profile_hook():
            return mod._hook

        mod.set_axon_ntff_profile_hook = set_axon_ntff_profile_hook
        mod.get_axon_ntff_profile_hook = get_axon_ntff_profile_hook
        sys.modules["antenv.axon_hooks"] = mod
        antenv.axon_hooks = mod
        hook = _ntff_profile_via_ctypes("/opt/axon/libaxon_pjrt.so")
        if hook is not None:
            set_axon_ntff_profile_hook(hook)
        return hook is not None
    except Exception as e:  # profiling is best-effort
        print(f"ntff hook install failed: {e}")
        return False


def _wrap16(stream_i16, n_tiles):
    """Wrapped+replicated dma_gather index layout: stream position i lives at
    partition i%16 (replicated to all 8 16-partition groups), slot i//16."""
    n = n_tiles * 128
    w = np.zeros((128, n // 16), dtype=np.int16)
    s = np.zeros(n, dtype=np.int16)
    s[: len(stream_i16)] = stream_i16
    blk = s.reshape(n // 16, 16).T  # [16, n//16]
    for g in range(8):
        w[g * 16 : (g + 1) * 16, :] = blk
    return w


def _prep(X, W, edge_row, edge_col, edge_vals):
    """Host-side sharding/marshalling.

    Returns (in_maps, T): per-window tile counts (maxed across cores so all
    8 cores run the identical SPMD program).
    """
    XT = np.ascontiguousarray(X.T).astype(BF16)  # [F_IN, N_NODES]
    Wb = np.ascontiguousarray(W).astype(BF16)  # [F_IN, F_OUT]
    # iota_big[p, r*GB + j] = r: a REAL (non-broadcast) operand for the
    # [128, WIN, GB]-layout one-hot build, so both tensor_tensor inputs
    # stream with unit inner stride.
    iota = np.tile(np.repeat(np.arange(WIN, dtype=np.float32), GB), (128, 1))

    n_win = (SHARD + WIN - 1) // WIN  # 49
    core = edge_row // SHARD
    percore = []
    cnt = np.zeros((N_CORES, n_win), dtype=np.int64)
    for p in range(N_CORES):
        m = core == p
        r = edge_row[m].astype(np.int64) - p * SHARD
        c = edge_col[m].astype(np.int64)
        v = edge_vals[m].astype(np.float32)
        w = r // WIN
        q = c % HALF  # table row (virtual row == table row in v4 layout)
        par = (c >= HALF).astype(np.float32)  # 0 -> cols 0:64, 1 -> 64:128
        # sort by (window, q): monotone gather addresses within each window
        # give the HBM controller sequential-ish access patterns.
        order = np.lexsort((q, w))
        r, q, v, par, w = r[order], q[order], v[order], par[order], w[order]
        percore.append((r, q, v, par, w))
        cnt[p] = np.bincount(w, minlength=n_win)

    T = np.maximum(1, -(-cnt.max(axis=0) // 128))
    J0 = int(T.sum())
    J = -(-J0 // CH) * CH  # pad to whole gather chunks
    starts = np.concatenate([[0], np.cumsum(T)])

    in_maps = []
    for p in range(N_CORES):
        r, q, v, par, w = percore[p]
        qbuf = np.zeros(J * 128, dtype=np.int64)
        rowloc = np.zeros(J * 128, dtype=np.float32)
        valsE = np.zeros(J * 128, dtype=np.float32)
        valsO = np.zeros(J * 128, dtype=np.float32)
        wstart = np.searchsorted(w, np.arange(n_win))
        wend = np.searchsorted(w, np.arange(n_win), side="right")
        for wi in range(n_win):
            a, b = int(wstart[wi]), int(wend[wi])
            n = b - a
            s0 = int(starts[wi]) * 128
            qbuf[s0 : s0 + n] = q[a:b]
            rowloc[s0 : s0 + n] = (r[a:b] % WIN).astype(np.float32)
            valsE[s0 : s0 + n] = v[a:b] * (1.0 - par[a:b])
            valsO[s0 : s0 + n] = v[a:b] * par[a:b]
        meta = np.concatenate(
            [
                rowloc.reshape(J, 128).T,
                valsE.reshape(J, 128).T,
                valsO.reshape(J, 128).T,
                iota,
            ],
            axis=1,
        ).astype(BF16)
        # phase-1 node slice for this core: lo nodes [3200p, 3200(p+1)),
        # hi nodes [HALF+3200p, HALF+3200(p+1)) zero-padded past N_NODES.
        lo0 = PT * 128 * p
        hi0 = HALF + PT * 128 * p
        xts = np.zeros((F_IN, 2 * PT * 128), dtype=BF16)
        xts[:, : PT * 128] = XT[:, lo0 : lo0 + PT * 128]
        nh = max(0, min(PT * 128, N_NODES - hi0))
        if nh > 0:
            xts[:, PT * 128 : PT * 128 + nh] = XT[:, hi0 : hi0 + nh]
        in_maps.append(
            {
                "xt": np.ascontiguousarray(xts),
                "w": Wb,
                "cols": _wrap16(qbuf.astype(np.int16), J),
                "meta": np.ascontiguousarray(meta),
            }
        )
    return in_maps, T


def _build_nc(T, f_in=F_IN, f_out=F_OUT, shard=SHARD):
    f32 = mybir.dt.float32
    bf16 = mybir.dt.bfloat16
    i16 = mybir.dt.int16
    n_win = len(T)
    J0 = int(T.sum())
    J = -(-J0 // CH) * CH
    starts = np.concatenate([[0], np.cumsum(T)])

    nc = bacc.Bacc(
        get_trn_type() or "TRN2",
        target_bir_lowering=False,
        dynamic_dma_scratch_size=32768,
        num_swdge_queues=NQ,
        num_devices=N_CORES,
    )
    xt = nc.dram_tensor("xt", [f_in, 2 * PT * 128], bf16, kind="ExternalInput")
    w_in = nc.dram_tensor("w", [f_in, f_out], bf16, kind="ExternalInput")
    cols = nc.dram_tensor("cols", [128, J * 8], i16, kind="ExternalInput")
    meta = nc.dram_tensor("meta", [128, 3 * J + GB * WIN], bf16, kind="ExternalInput")
    out = nc.dram_tensor("out", [shard, f_out], f32, kind="ExternalOutput")
    # XW table: row r packs node r (cols 0:64) and node r+HALF (cols 64:128).
    # Table row r = a*128 + b lives at staging partition b, col (a*128+f),
    # so each core's 25-pair-tile block is a contiguous 0.8 MB row range and
    # the 8 blocks AllGather straight into place.
    xw_loc = nc.dram_tensor("xw_loc", [PT * 128, 128], bf16, kind="Internal")
    xw = nc.dram_tensor("xw", [HALF, 128], bf16, kind="Internal")

    n_kc = f_in // 128  # contraction chunks (2)

    with tile.TileContext(nc) as tc, ExitStack() as ctx:
        const = ctx.enter_context(tc.tile_pool(name="const", bufs=1))
        psum1 = ctx.enter_context(tc.tile_pool(name="psum1", bufs=4, space="PSUM"))
        gath = [
            ctx.enter_context(tc.tile_pool(name=f"gath{qi}", bufs=2))
            for qi in range(NQ)
        ]
        s_pool = ctx.enter_context(tc.tile_pool(name="s_pool", bufs=3))
        se_pool = ctx.enter_context(tc.tile_pool(name="se_pool", bufs=3))
        so_pool = ctx.enter_context(tc.tile_pool(name="so_pool", bufs=3))
        psum2 = ctx.enter_context(tc.tile_pool(name="psum2", bufs=4, space="PSUM"))
        out_sb = ctx.enter_context(tc.tile_pool(name="out_sb", bufs=4))

        # resident constants
        w_t = []
        for k in range(n_kc):
            wt = const.tile([128, f_out], bf16, tag=f"w{k}")
            nc.sync.dma_start(out=wt[:], in_=w_in[k * 128 : (k + 1) * 128, :])
            w_t.append(wt)
        meta_t = const.tile([128, 3 * J + GB * WIN], bf16, tag="meta")
        nc.sync.dma_start(out=meta_t[:], in_=meta[:, :])
        cols_t = const.tile([128, J * 8], i16, tag="cols")
        nc.sync.dma_start(out=cols_t[:], in_=cols[:, :])

        # shared num_idxs register for all gather calls (one MOVE total)
        nreg = nc.gpsimd.to_reg(CH * 128)

        # ---- phase 1 (sharded): this core's 25 pair-tiles, then AllGather ----
        xtl = []
        for k in range(n_kc):
            t1 = const.tile([128, 2 * PT * 128], bf16, tag=f"xts{k}")
            nc.sync.dma_start(out=t1[:], in_=xt[k * 128 : (k + 1) * 128, :])
            xtl.append(t1)
        stg = const.tile([128, PT * 128], bf16, tag="stg")
        for i in range(PT):
            ps = psum1.tile([128, 128], f32, tag="ps1")
            for k in range(n_kc):
                nc.tensor.matmul(
                    out=ps[:, 0:f_out],
                    lhsT=xtl[k][:, i * 128 : (i + 1) * 128],
                    rhs=w_t[k][:],
                    start=(k == 0),
                    stop=(k == n_kc - 1),
                )
            for k in range(n_kc):
                nc.tensor.matmul(
                    out=ps[:, f_out:128],
                    lhsT=xtl[k][:, PT * 128 + i * 128 : PT * 128 + (i + 1) * 128],
                    rhs=w_t[k][:],
                    start=(k == 0),
                    stop=(k == n_kc - 1),
                )
            dst = stg[:, i * 128 : (i + 1) * 128]
            if i % 2 == 0:
                nc.scalar.activation(
                    out=dst, in_=ps[:], func=mybir.ActivationFunctionType.Copy
                )
            else:
                nc.vector.tensor_copy(out=dst, in_=ps[:])
        # stg partition b, col (a*128+f)  ->  xw_loc row a*128+b, col f
        xwl_3 = xw_loc[:, :].rearrange("(a b) f -> b a f", b=128)  # [128, PT, 128]
        nc.sync.dma_start(
            out=xwl_3[:, :, :],
            in_=stg[:, :].rearrange("b (a f) -> b a f", f=128),
        )
        nc.gpsimd.collective_compute(
            "AllGather",
            mybir.AluOpType.bypass,
            replica_groups=[list(range(N_CORES))],
            ins=[xw_loc[:, :]],
            outs=[xw[:, :]],
        )

        # ---- phase 2: multi-queue dma_gather + one-hot matmul segment-sum ----
        chunks = {}
        batches = {}

        def ensure_chunk(tile_idx):
            ci = tile_idx // CH
            if ci in chunks:
                return chunks[ci]
            q = ci % NQ
            g = gath[q].tile([128, CH, 128], bf16, tag=f"g{q}")
            nc.gpsimd.dma_gather(
                out_ap=g[:, :, :],
                in_ap=xw[:, :],
                idxs_ap=cols_t[:, ci * CH * 8 : (ci + 1) * CH * 8],
                num_idxs=CH * 128,
                num_idxs_reg=nreg,
                elem_size=128,
                single_packet=False,
                queue_num=q,
            )
            chunks[ci] = g
            return g

        def ensure_batch(tile_idx):
            bi = tile_idx // GB
            if bi in batches:
                return batches[bi]
            b0 = bi * GB
            S_b = s_pool.tile([128, WIN, GB], bf16, tag="S")
            SE_b = se_pool.tile([128, WIN, GB], bf16, tag="SE")
            SO_b = so_pool.tile([128, WIN, GB], bf16, tag="SO")
            nc.vector.tensor_tensor(
                out=S_b[:],
                in0=meta_t[:, 3 * J : 3 * J + WIN * GB].rearrange(
                    "p (r b) -> p r b", b=GB
                ),
                in1=meta_t[:, b0 : b0 + GB]
                .rearrange("p (one b) -> p one b", one=1)
                .to_broadcast([128, WIN, GB]),
                op=mybir.AluOpType.is_equal,
            )
            nc.vector.tensor_tensor(
                out=SE_b[:],
                in0=S_b[:],
                in1=meta_t[:, J + b0 : J + b0 + GB]
                .rearrange("p (one b) -> p one b", one=1)
                .to_broadcast([128, WIN, GB]),
                op=mybir.AluOpType.mult,
            )
            nc.vector.tensor_tensor(
                out=SO_b[:],
                in0=S_b[:],
                in1=meta_t[:, 2 * J + b0 : 2 * J + b0 + GB]
                .rearrange("p (one b) -> p one b", one=1)
                .to_broadcast([128, WIN, GB]),
                op=mybir.AluOpType.mult,
            )
            batches[bi] = (SE_b, SO_b)
            return batches[bi]

        for w in range(n_win):
            cur_ps = psum2.tile([128, f_out], f32, tag="ps2")
            n_t = int(T[w])
            for k in range(n_t):
                t_s = int(starts[w]) + k
                SE_b, SO_b = ensure_batch(t_s)
                g = ensure_chunk(t_s)
                sl = t_s % GB
                gs = t_s - (t_s // CH) * CH
                nc.tensor.matmul(
                    out=cur_ps[:],
                    lhsT=SE_b[:, :, sl : sl + 1].rearrange("p r one -> p (r one)"),
                    rhs=g[:, gs : gs + 1, 0:f_out],
                    start=(k == 0),
                    stop=False,
                )
                nc.tensor.matmul(
                    out=cur_ps[:],
                    lhsT=SO_b[:, :, sl : sl + 1].rearrange("p r one -> p (r one)"),
                    rhs=g[:, gs : gs + 1, f_out:128],
                    start=False,
                    stop=(k == n_t - 1),
                )
            rows = min(WIN, shard - w * WIN)
            ot = out_sb.tile([128, f_out], f32, tag="ot")
            nc.scalar.activation(
                out=ot[:rows, :],
                in_=cur_ps[:rows, :],
                func=mybir.ActivationFunctionType.Copy,
            )
            nc.sync.dma_start(out=out[w * WIN : w * WIN + rows, :], in_=ot[:rows, :])
    nc.compile()
    return nc


def kernel(X, W, edge_row, edge_col, edge_vals):
    global LAST_RESULTS
    X = np.asarray(X, dtype=np.float32)
    W = np.asarray(W, dtype=np.float32)
    edge_row = np.asarray(edge_row, dtype=np.int32)
    edge_col = np.asarray(edge_col, dtype=np.int32)
    edge_vals = np.asarray(edge_vals, dtype=np.float32)

    in_maps, T = _prep(X, W, edge_row, edge_col, edge_vals)
    nc = _build_nc(T)
    trace = TRACE and _install_ntff_hook()
    res = run_bass_kernel_spmd(
        nc, in_maps, core_ids=list(range(N_CORES)), trace=trace
    )
    LAST_RESULTS = res
    out = np.concatenate([res.results[p]["out"] for p in range(N_CORES)], axis=0)
    return out.astype(np.float32)
